# revision 15
# baseline (speedup 1.0000x reference)
"""Per-edge dot product kernel for Trainium2 (8 NeuronCores).

Computes out[e] = sum(h[src[e]] * h[dst[e]], axis=-1) for
h: [100000, 64] f32, src/dst: [1000000] int indices.

Design (v4 -- big-chunk SWDGE gathers, input-baked sizes):
  - Edges sharded across 8 cores (125k each). h kept in HBM as
    [4, 25000, 64] f32 pieces (dma_gather idx is int16, so gathers
    address one 25000-row piece).
  - Host buckets each core's edges by (src piece, dst piece) -- 16
    buckets. v3 used fixed 896-edge chunks (288 gathers/core); the
    trace showed Pool-engine desc-gen serialized at ~994ns fixed +
    ~1.3ns/row per dma_gather, so the fixed cost was ~45% of the
    total. v4 bakes the input's actual bucket sizes into the program
    (compile is per-input) and uses ONE chunk per bucket (up to
    CH_MAX rows; HW caps num_idxs at 1024/gather). Cores are padded
    with dummy idx-0 edges up to the max-over-cores bucket count so
    the SPMD program's static num_idxs matches every core (~1-2%
    row overhead).
  - Each chunk issues two dma_gather instructions (src piece, dst
    piece) round-robined over the 4 SWDGE queues. Gathered rows land
    edge-major [128, M, 64] f32 (row i -> partition i%128, slot
    i//128).
  - DVE multiplies and reduces over D=64 -> dots [128, M] f32,
    DMA'd out per chunk. Full f32 precision end to end.
  - Host unsorts chunk slots back to edge order (index bookkeeping
    only; all FLOPs and h movement happen on device).
"""

import sys

import numpy as np

_TRN_REPO = "/opt/trn_rl_repo"
if _TRN_REPO not in sys.path:
    sys.path.insert(0, _TRN_REPO)

N_NODES = 100000
N_EDGES = 1000000
D = 64
N_CORES = 8
E_CORE = N_EDGES // N_CORES   # 125000

NPIECE = 4
WPIECE = 25000                # 4 * 25000 = 100000
NBUCKET = NPIECE * NPIECE     # 16
CH_MAX = 992                  # ring limit: num_idxs/16+1 descs/dma <= 1024

_PROGRAM_CACHE = {}


def _plan_chunks(src, dst):
    """Bucket counts -> per-bucket (n_chunks, chunk_size) baked into the
    program. chunk_size is a multiple of 128; capacity >= max-over-cores
    bucket count (cores pad with dummy edges up to capacity)."""
    cmax = np.zeros(NBUCKET, dtype=np.int64)
    for c in range(N_CORES):
        sl = slice(c * E_CORE, (c + 1) * E_CORE)
        bb = (src[sl] // WPIECE) * NPIECE + dst[sl] // WPIECE
        cmax = np.maximum(cmax, np.bincount(bb, minlength=NBUCKET))
    plan = []
    for b in range(NBUCKET):
        n = max(1, -(-int(cmax[b]) // CH_MAX))
        s = -(-int(cmax[b]) // (n * 128)) * 128
        plan.append((n, s))
    return tuple(plan)


def _build_program(plan):
    import concourse.tile as tile
    from concourse import bacc, mybir

    nc = bacc.Bacc("TRN2", target_bir_lowering=False, debug=False,
                   num_swdge_queues=4)

    h_t = nc.dram_tensor("h4", [NPIECE, WPIECE, D], mybir.dt.float32,
                         kind="ExternalInput")
    si_ts, di_ts, out_ts = [], [], []
    for b, (n, s) in enumerate(plan):
        si_ts.append(nc.dram_tensor(f"src_idx{b}", [n, 128, s // 16],
                                    mybir.dt.int16, kind="ExternalInput"))
        di_ts.append(nc.dram_tensor(f"dst_idx{b}", [n, 128, s // 16],
                                    mybir.dt.int16, kind="ExternalInput"))
        out_ts.append(nc.dram_tensor(f"edot{b}", [n, 128, s // 128],
                                     mybir.dt.float32, kind="ExternalOutput"))

    with tile.TileContext(nc) as tc:
        with (
            tc.tile_pool(name="idx", bufs=8) as ipool,
            tc.tile_pool(name="gat", bufs=4) as gpool,
            tc.tile_pool(name="prd", bufs=2) as ppool,
            tc.tile_pool(name="dot", bufs=4) as dpool,
        ):
            q = 0
            for b, (n, s) in enumerate(plan):
                ps, pd = divmod(b, NPIECE)
                m = s // 128
                for j in range(n):
                    si = ipool.tile([128, s // 16], mybir.dt.int16, tag="si")
                    nc.sync.dma_start(out=si[:], in_=si_ts[b].ap()[j])
                    di = ipool.tile([128, s // 16], mybir.dt.int16, tag="di")
                    nc.scalar.dma_start(out=di[:], in_=di_ts[b].ap()[j])

                    hs = gpool.tile([128, m, D], mybir.dt.float32, tag="hs")
                    nc.gpsimd.dma_gather(
                        out_ap=hs[:], in_ap=h_t.ap()[ps], idxs_ap=si[:],
                        num_idxs=s, num_idxs_reg=s, elem_size=D,
                        queue_num=q % 4,
                    )
                    q += 1
                    hd = gpool.tile([128, m, D], mybir.dt.float32, tag="hd")
                    nc.gpsimd.dma_gather(
                        out_ap=hd[:], in_ap=h_t.ap()[pd], idxs_ap=di[:],
                        num_idxs=s, num_idxs_reg=s, elem_size=D,
                        queue_num=q % 4,
                    )
                    q += 1

                    prod = ppool.tile([128, m, D], mybir.dt.float32,
                                      tag="prod")
                    nc.vector.tensor_mul(
                        out=prod[:].rearrange("p m d -> p (m d)"),
                        in0=hs[:].rearrange("p m d -> p (m d)"),
                        in1=hd[:].rearrange("p m d -> p (m d)"),
                    )
                    dots = dpool.tile([128, m], mybir.dt.float32, tag="dots")
                    nc.vector.tensor_reduce(
                        out=dots[:],
                        in_=prod[:],
                        axis=mybir.AxisListType.X,
                        op=mybir.AluOpType.add,
                    )
                    nc.sync.dma_start(out=out_ts[b].ap()[j], in_=dots[:])

    nc.compile()
    return nc


def _get_program(plan):
    if plan not in _PROGRAM_CACHE:
        _PROGRAM_CACHE[plan] = _build_program(plan)
    return _PROGRAM_CACHE[plan]


def _prep_core(src, dst, plan):
    """Pack one core's edges per the chunk plan. Returns (in_map,
    (bucket, chunk, part, mslot) per edge for output reconstruction)."""
    ps = src // WPIECE
    pd = dst // WPIECE
    b = ps * NPIECE + pd
    order = np.argsort(b, kind="stable")
    counts = np.bincount(b, minlength=NBUCKET)

    pos = np.empty(len(src), dtype=np.int64)   # rank within bucket
    off = np.concatenate([[0], np.cumsum(counts)])
    pos[order] = np.arange(len(src)) - off[b[order]]

    src_local = (src - ps * WPIECE).astype(np.int16)
    dst_local = (dst - pd * WPIECE).astype(np.int16)

    in_map = {}
    # edge -> (bucket, chunk, slot)
    s_of_b = np.array([s for _, s in plan], dtype=np.int64)
    chunk = pos // s_of_b[b]
    i = pos % s_of_b[b]

    for bb, (n, s) in enumerate(plan):
        # dummy idx-0 padding up to full capacity (all cores identical
        # static num_idxs; dummies gather piece row 0, output discarded)
        src_idx = np.zeros((n, 128, s // 16), dtype=np.int16)
        dst_idx = np.zeros((n, 128, s // 16), dtype=np.int16)
        sel = b == bb
        ii = i[sel]
        # wrapped layout: slot i -> [16*core + i%16, i//16], replicated
        # across the 8 q7 core groups
        prow = ii % 16
        pfree = ii // 16
        for corep in range(8):
            src_idx[chunk[sel], 16 * corep + prow, pfree] = src_local[sel]
            dst_idx[chunk[sel], 16 * corep + prow, pfree] = dst_local[sel]
        in_map[f"src_idx{bb}"] = src_idx
        in_map[f"dst_idx{bb}"] = dst_idx

    return in_map, (b, chunk, i % 128, i // 128)


def _run(h, src, dst, trace=False):
    from concourse.bass_utils import run_bass_kernel_spmd

    h = np.ascontiguousarray(np.asarray(h, dtype=np.float32))
    src = np.asarray(src).astype(np.int64)
    dst = np.asarray(dst).astype(np.int64)

    h4 = h.reshape(NPIECE, WPIECE, D)
    plan = _plan_chunks(src, dst)

    in_maps = []
    metas = []
    for c in range(N_CORES):
        sl = slice(c * E_CORE, (c + 1) * E_CORE)
        in_map, meta = _prep_core(src[sl], dst[sl], plan)
        in_map["h4"] = h4
        metas.append(meta)
        in_maps.append(in_map)

    nc = _get_program(plan)
    res = run_bass_kernel_spmd(nc, in_maps, list(range(N_CORES)), trace=trace)

    parts = []
    for c in range(N_CORES):
        bkt, chunk, part, mslot = metas[c]
        vals = np.empty(E_CORE, dtype=np.float32)
        for bb in range(NBUCKET):
            dots = np.asarray(res.results[c][f"edot{bb}"])  # [n, 128, m]
            sel = bkt == bb
            vals[sel] = dots[chunk[sel], part[sel], mslot[sel]]
        parts.append(vals)
    return np.concatenate(parts), res


def kernel(h, src, dst):
    out, _ = _run(h, src, dst)
    return out


# revision 17
# speedup vs baseline: 1.0356x; 1.0356x over previous
"""Per-edge dot product kernel for Trainium2 (8 NeuronCores).

Computes out[e] = sum(h[src[e]] * h[dst[e]], axis=-1) for
h: [100000, 64] f32, src/dst: [1000000] int indices.

Design (v4 -- big-chunk SWDGE gathers, input-baked sizes):
  - Edges sharded across 8 cores (125k each). h kept in HBM as
    [4, 25000, 64] f32 pieces (dma_gather idx is int16, so gathers
    address one 25000-row piece).
  - Host buckets each core's edges by (src piece, dst piece) -- 16
    buckets. v3 used fixed 896-edge chunks (288 gathers/core); the
    trace showed Pool-engine desc-gen serialized at ~994ns fixed +
    ~1.3ns/row per dma_gather, so the fixed cost was ~45% of the
    total. v4 bakes the input's actual bucket sizes into the program
    (compile is per-input) and uses ONE chunk per bucket (up to
    CH_MAX rows; HW caps num_idxs at 1024/gather). Cores are padded
    with dummy idx-0 edges up to the max-over-cores bucket count so
    the SPMD program's static num_idxs matches every core (~1-2%
    row overhead).
  - Each chunk issues two dma_gather instructions (src piece, dst
    piece) round-robined over the 4 SWDGE queues. Gathered rows land
    edge-major [128, M, 64] f32 (row i -> partition i%128, slot
    i//128).
  - DVE multiplies and reduces over D=64 -> dots [128, M] f32,
    DMA'd out per chunk. Full f32 precision end to end.
  - Host unsorts chunk slots back to edge order (index bookkeeping
    only; all FLOPs and h movement happen on device).
"""

import sys

import numpy as np

_TRN_REPO = "/opt/trn_rl_repo"
if _TRN_REPO not in sys.path:
    sys.path.insert(0, _TRN_REPO)

N_NODES = 100000
N_EDGES = 1000000
D = 64
N_CORES = 8
E_CORE = N_EDGES // N_CORES   # 125000

NPIECE = 4
WPIECE = 25000                # 4 * 25000 = 100000
NBUCKET = NPIECE * NPIECE     # 16
CH_MAX = 992                  # ring limit: num_idxs/16+1 descs/dma <= 1024

_PROGRAM_CACHE = {}


def _plan_chunks(src, dst):
    """Bucket counts -> per-bucket (n_chunks, chunk_size) baked into the
    program. chunk_size is a multiple of 128; capacity >= max-over-cores
    bucket count (cores pad with dummy edges up to capacity)."""
    cmax = np.zeros(NBUCKET, dtype=np.int64)
    for c in range(N_CORES):
        sl = slice(c * E_CORE, (c + 1) * E_CORE)
        bb = (src[sl] // WPIECE) * NPIECE + dst[sl] // WPIECE
        cmax = np.maximum(cmax, np.bincount(bb, minlength=NBUCKET))
    plan = []
    for b in range(NBUCKET):
        n = max(1, -(-int(cmax[b]) // CH_MAX))
        s = -(-int(cmax[b]) // (n * 128)) * 128
        plan.append((n, s))
    return tuple(plan)


def _build_program(plan):
    import concourse.tile as tile
    from concourse import bacc, mybir

    nc = bacc.Bacc("TRN2", target_bir_lowering=False, debug=False,
                   num_swdge_queues=4)

    h_t = nc.dram_tensor("h4", [NPIECE, WPIECE, D], mybir.dt.float32,
                         kind="ExternalInput")
    si_ts, di_ts, out_ts = [], [], []
    for b, (n, s) in enumerate(plan):
        si_ts.append(nc.dram_tensor(f"src_idx{b}", [n, 128, s // 16],
                                    mybir.dt.int16, kind="ExternalInput"))
        di_ts.append(nc.dram_tensor(f"dst_idx{b}", [n, 128, s // 16],
                                    mybir.dt.int16, kind="ExternalInput"))
        out_ts.append(nc.dram_tensor(f"edot{b}", [n, 128, s // 128],
                                     mybir.dt.float32, kind="ExternalOutput"))

    with tile.TileContext(nc) as tc:
        with (
            tc.tile_pool(name="idx", bufs=8) as ipool,
            tc.tile_pool(name="gat", bufs=4) as gpool,
            tc.tile_pool(name="prd", bufs=2) as ppool,
            tc.tile_pool(name="dot", bufs=4) as dpool,
        ):
            q = 0
            for b, (n, s) in enumerate(plan):
                ps, pd = divmod(b, NPIECE)
                m = s // 128
                for j in range(n):
                    si = ipool.tile([128, s // 16], mybir.dt.int16, tag="si")
                    nc.sync.dma_start(out=si[:], in_=si_ts[b].ap()[j])
                    di = ipool.tile([128, s // 16], mybir.dt.int16, tag="di")
                    nc.scalar.dma_start(out=di[:], in_=di_ts[b].ap()[j])

                    hs = gpool.tile([128, m, D], mybir.dt.float32, tag="hs")
                    nc.gpsimd.dma_gather(
                        out_ap=hs[:], in_ap=h_t.ap()[ps], idxs_ap=si[:],
                        num_idxs=s, num_idxs_reg=s, elem_size=D,
                        queue_num=q % 4,
                    )
                    q += 1
                    hd = gpool.tile([128, m, D], mybir.dt.float32, tag="hd")
                    nc.gpsimd.dma_gather(
                        out_ap=hd[:], in_ap=h_t.ap()[pd], idxs_ap=di[:],
                        num_idxs=s, num_idxs_reg=s, elem_size=D,
                        queue_num=q % 4,
                    )
                    q += 1

                    prod = ppool.tile([128, m, D], mybir.dt.float32,
                                      tag="prod")
                    nc.vector.tensor_mul(
                        out=prod[:].rearrange("p m d -> p (m d)"),
                        in0=hs[:].rearrange("p m d -> p (m d)"),
                        in1=hd[:].rearrange("p m d -> p (m d)"),
                    )
                    dots = dpool.tile([128, m], mybir.dt.float32, tag="dots")
                    nc.vector.tensor_reduce(
                        out=dots[:],
                        in_=prod[:],
                        axis=mybir.AxisListType.X,
                        op=mybir.AluOpType.add,
                    )
                    nc.sync.dma_start(out=out_ts[b].ap()[j], in_=dots[:])

    nc.compile()
    return nc


def _get_program(plan):
    if plan not in _PROGRAM_CACHE:
        _PROGRAM_CACHE[plan] = _build_program(plan)
    return _PROGRAM_CACHE[plan]


def _prep_core(src, dst, plan):
    """Pack one core's edges per the chunk plan. Returns (in_map,
    (bucket, chunk, part, mslot) per edge for output reconstruction)."""
    ps = src // WPIECE
    pd = dst // WPIECE
    b = ps * NPIECE + pd
    order = np.argsort(b, kind="stable")
    counts = np.bincount(b, minlength=NBUCKET)

    pos = np.empty(len(src), dtype=np.int64)   # rank within bucket
    off = np.concatenate([[0], np.cumsum(counts)])
    pos[order] = np.arange(len(src)) - off[b[order]]

    src_local = (src - ps * WPIECE).astype(np.int16)
    dst_local = (dst - pd * WPIECE).astype(np.int16)

    in_map = {}
    # edge -> (bucket, chunk, slot)
    s_of_b = np.array([s for _, s in plan], dtype=np.int64)
    chunk = pos // s_of_b[b]
    i = pos % s_of_b[b]

    for bb, (n, s) in enumerate(plan):
        # dummy idx-0 padding up to full capacity (all cores identical
        # static num_idxs; dummies gather piece row 0, output discarded)
        src_idx = np.zeros((n, 128, s // 16), dtype=np.int16)
        dst_idx = np.zeros((n, 128, s // 16), dtype=np.int16)
        sel = b == bb
        ii = i[sel]
        # wrapped layout: slot i -> [16*core + i%16, i//16], replicated
        # across the 8 q7 core groups
        prow = ii % 16
        pfree = ii // 16
        for corep in range(8):
            src_idx[chunk[sel], 16 * corep + prow, pfree] = src_local[sel]
            dst_idx[chunk[sel], 16 * corep + prow, pfree] = dst_local[sel]
        in_map[f"src_idx{bb}"] = src_idx
        in_map[f"dst_idx{bb}"] = dst_idx

    return in_map, (b, chunk, i % 128, i // 128)


def _run(h, src, dst, trace=False):
    from concourse.bass_utils import run_bass_kernel_spmd

    h = np.ascontiguousarray(np.asarray(h, dtype=np.float32))
    src = np.asarray(src).astype(np.int64)
    dst = np.asarray(dst).astype(np.int64)

    h4 = h.reshape(NPIECE, WPIECE, D)
    plan = _plan_chunks(src, dst)

    in_maps = []
    metas = []
    for c in range(N_CORES):
        sl = slice(c * E_CORE, (c + 1) * E_CORE)
        in_map, meta = _prep_core(src[sl], dst[sl], plan)
        in_map["h4"] = h4
        metas.append(meta)
        in_maps.append(in_map)

    nc = _get_program(plan)
    res = run_bass_kernel_spmd(nc, in_maps, list(range(N_CORES)), trace=trace)

    parts = []
    for c in range(N_CORES):
        bkt, chunk, part, mslot = metas[c]
        vals = np.empty(E_CORE, dtype=np.float32)
        for bb in range(NBUCKET):
            dots = np.asarray(res.results[c][f"edot{bb}"])  # [n, 128, m]
            sel = bkt == bb
            vals[sel] = dots[chunk[sel], part[sel], mslot[sel]]
        parts.append(vals)
    return np.concatenate(parts), res


def kernel(h, src, dst):
    out, _ = _run(h, src, dst)
    return out
